# revision 41
# baseline (speedup 1.0000x reference)
"""Trainium2 Bass kernel for nn_BCHConv2D (complex harmonic conv + bispectrum).

Strategy (8 NeuronCores, data-parallel over batch B=8):
  host: build complex-harmonic filters from w+atoms -> fdA [128, 7*208],
        fdB [96, 7*208] (kh-major along free dim); transpose each batch
        image to (H, C, W); un-swap output row pairs at the end.
  core, conv: 7x7x32 -> 208ch conv with ROW-PAIRED psum banks: one bank
        holds two adjacent output rows [122, 2*208]; a single matmul
        streams a 416-col moving slice fd[kh-1 | kh] accumulating both
        rows at once (8 matmuls + 8 LDWEIGHTS per pair-side instead of
        14 per-row), with a full-width start=True matmul first (psum
        start resets the whole bank) and the even row's kh=0 single
        deferred after it. PE warms up ~28 matmuls on random SBUF data
        immediately (no DMA dependency; HAM pstate ramps while the
        filter/stack DMAs land). Conv runs at ~97% of its structural
        floor (14x208 stream cols/row, fp16 1 cyc/row).
  core, bisp: PSUM pair -> fp16 SBUF copy (one ACT copy per pair) ->
        bispectrum as blocked DVE ops (+ ACT squares) over ~10-row
        groups (schedule-DP optimum: small groups keep the DVE fed as
        conv rows arrive, shrinking the after-conv Vector backlog) ->
        stride-0-broadcast bias add + relu -> output DMA.
  DMA:  input stacks on sync; outputs on gpsimd (its software DGE
        spreads DRAM-write descriptors across the engine fleet; HW
        queues pin writes to ~2 engines at ~11GB/s). Last two groups
        stream out in 2-row chunks, the final chunk as two 1-row DMAs
        on idle queues so the end-of-kernel drain is short.
"""
import os
import sys
import types
from itertools import product

import numpy as np

sys.path.insert(0, "/opt/trn_rl_repo")
sys.path.insert(0, "/root/.axon_site")

import concourse.bass as bass
import concourse.bacc as bacc
import concourse.tile as tile
from concourse import mybir
from concourse import bass_utils

# ---------------- problem constants ----------------
KS, MD, STREAMS, C_IN = 7, 6, 16, 32
H = W = 128
HO = WO = 122
NC_RE = (MD + 1) * STREAMS       # 112
NCONV = 208                      # re[0..6] (112) + im[1..6] (96); im0 == 0
IM_BASE = NC_RE - 16             # im[n] at IM_BASE + n*16 for n >= 1
NB = 8                           # batch == cores
NPAIR = HO // 2                  # 61 row pairs

F16 = mybir.dt.float16
F32 = mybir.dt.float32

NWARM = int(os.environ.get("NWARM", "28"))


# ---------------- host-side filter construction ----------------
def _tri(v):
    return np.where(np.abs(v) <= 1, np.where(v < 0, v + 1, 1 - v), 0)


def _make_atoms(kernel_size, max_degree):
    radius = (kernel_size - 1) // 2
    g = np.arange(-radius, radius + 1)
    xg, yg = np.meshgrid(g, g)
    r = np.sqrt(xg ** 2 + yg ** 2)
    theta = np.arctan2(yg, xg)
    n_rp = kernel_size // 2 + 1
    atoms = np.zeros((kernel_size, kernel_size, max_degree + 1, n_rp),
                     dtype=np.complex64)
    for i, n in product(range(n_rp), range(max_degree + 1)):
        atoms[:, :, n, i] = _tri(r - i) * np.exp(theta * n * -1j)
    atoms[kernel_size // 2, kernel_size // 2, 1:, :] = 0
    norm = np.sqrt(np.sum(np.conj(atoms) * atoms, axis=(0, 1)))
    norm[norm == 0] = 1
    return (atoms / norm).astype(np.complex64)


_ATOMS = _make_atoms(KS, MD)


def _host_filters(w):
    """w (1,1,32,16,7,4) -> (fdA [128, 7*208], fdB [96, 7*208]) float16.
    Conv channel order: col n*16+s = re(n,s); 112+(n-1)*16+s = im(n,s)."""
    wc = w[0, 0]
    f_re = np.einsum("hwnr,csnr->hwcsn", _ATOMS.real, wc)
    f_im = np.einsum("hwnr,csnr->hwcsn", _ATOMS.imag, wc)
    filt = np.zeros((KS, KS, C_IN, NCONV), np.float32)
    filt[:, :, :, 0:NC_RE] = np.transpose(f_re, (0, 1, 2, 4, 3)).reshape(
        KS, KS, C_IN, NC_RE)
    filt[:, :, :, NC_RE:NCONV] = np.transpose(
        f_im[:, :, :, :, 1:], (0, 1, 2, 4, 3)).reshape(KS, KS, C_IN, 96)
    # [kh, (kw,c), o] -> partition-major stacks, kh-major along free dim
    fa = filt[:, 0:4].reshape(KS, 128, NCONV).transpose(1, 0, 2).reshape(
        128, KS * NCONV)
    fb = filt[:, 4:7].reshape(KS, 96, NCONV).transpose(1, 0, 2).reshape(
        96, KS * NCONV)
    return fa.astype(np.float16), fb.astype(np.float16)


# ---------------- bass program ----------------
def _ap(src_ap, off, dims):
    """New AP into the same tensor: explicit [step, count] dims (elements)."""
    return bass.AP(tensor=src_ap.tensor, offset=src_ap.offset + off, ap=dims)


_PROGRAM = None

# bisp groups in row units: (h0, Rg); all starts/sizes even.
# ~10-row groups keep the DVE fed as conv rows arrive (schedule-DP
# optimum): the final group starts at conv-end either way, so smaller
# mid-kernel groups shrink the end backlog.
GDEF = [(0, 12)] + [(12 + 10 * k, 10) for k in range(11)]


def _build_program():
    nc = bacc.Bacc("TRN2", target_bir_lowering=False, debug=False,
                   num_devices=NB)
    xt_d = nc.dram_tensor("xt", [H, C_IN, W], F16, kind="ExternalInput").ap()
    fa_d = nc.dram_tensor("fa", [128, KS * NCONV], F16,
                          kind="ExternalInput").ap()
    fb_d = nc.dram_tensor("fb", [96, KS * NCONV], F16,
                          kind="ExternalInput").ap()
    bias_d = nc.dram_tensor("biasrep", [128, 256], F16,
                            kind="ExternalInput").ap()
    # w-major output layout: row pairs stored swapped (host un-swaps)
    out_d = nc.dram_tensor("out", [WO, HO, 256], F16,
                           kind="ExternalOutput").ap()
    MUL = mybir.AluOpType.mult
    ADD = mybir.AluOpType.add
    SUB = mybir.AluOpType.subtract

    with tile.TileContext(nc) as tc:
        with tc.tile_pool(name="const", bufs=1) as constp, \
             tc.tile_pool(name="stk", bufs=5) as stkp, \
             tc.tile_pool(name="fm", bufs=4) as fmp, \
             tc.tile_pool(name="tmp", bufs=2) as tmpp, \
             tc.tile_pool(name="yp", bufs=4) as yp, \
             tc.tile_pool(name="ps", bufs=8, space="PSUM") as psp:

            # ---- PE warmup on random SBUF data: no DMA dependency ----
            wtile = constp.tile([128, 256], F16, name="warm")
            nc.vector.random(wtile[:])
            wps = psp.tile([128, 416], F32, tag="ps", name="warmps")
            for _ in range(NWARM):
                nc.tensor.matmul(wps[0:122, 0:208], wtile[:, 0:122],
                                 wtile[:, 0:208], start=True, stop=True)

            # ---- constants (DMAs spread across queues) ----
            fA = constp.tile([128, KS * NCONV], F16, name="fA")
            fB = constp.tile([96, KS * NCONV], F16, name="fB")
            # heads first: exactly what rows 0-3 of the conv need (fd[0..1]),
            # so the warmup can be short; tails + bias follow the first
            # stack block (emitted below) so they don't delay it.
            HD = 2 * NCONV  # 416
            nc.scalar.dma_start(fA[:, 0:HD], fa_d[:, 0:HD])
            nc.scalar.dma_start(fB[:, 0:HD], fb_d[:, 0:HD])
            biasT = constp.tile([128, 256], F16)

            group = {}   # current bispectrum group state
            H2G = {}
            for gi, (s, n) in enumerate(GDEF):
                for j in range(n):
                    H2G[s + j] = (gi, j)

            def sview(t, off, dims, nparts=WO):
                a = t[:]
                return bass.AP(tensor=a.tensor, offset=a.offset + off,
                               ap=[[a.ap[0][0], nparts]] + dims)

            def bisp(fmT, h0, Rg, late=False):
                """Bispectrum for Rg rows; fm blocks hold rows pair-swapped
                (host un-swaps). Reads via run-decomposed broadcast APs."""
                def fmr(comp, n0, cnt):     # contiguous n-run view
                    return sview(fmT, comp + n0 * 16,
                                 [[NCONV, Rg], [1, cnt * 16]])

                def fmb(comp, n, reps):     # broadcast single-n view
                    return sview(fmT, comp + n * 16,
                                 [[NCONV, Rg], [0, reps], [1, 16]])

                t1 = tmpp.tile([128, 16 * 96], F16, tag="t1")
                t2 = tmpp.tile([128, 16 * 96], F16, tag="t2")
                t3 = tmpp.tile([128, 16 * 96], F16, tag="t3")
                t4 = tmpp.tile([128, 16 * 96], F16, tag="t4")

                def tv(t, p0, L):
                    return sview(t, p0 * 16, [[96, Rg], [1, L]])
                full = lambda t: t[0:WO, 0:Rg * 96]
                RE, IM = 0, IM_BASE
                # runs: (pair0, cnt, nA, nB0, nC0)
                RUNS = [(0, 3, 1, 1, 2), (3, 2, 2, 2, 4), (5, 1, 3, 3, 6)]
                # stage 1: re1 = ArBr - AiBi -> t1 ; im1 = ArBi + AiBr -> t3
                for (p0, cnt, na, nb, ncn) in RUNS:
                    L = cnt * 16
                    nc.vector.tensor_tensor(tv(t1, p0, L), fmb(RE, na, cnt),
                                            fmr(RE, nb, cnt), MUL)
                    nc.vector.tensor_tensor(tv(t2, p0, L), fmb(IM, na, cnt),
                                            fmr(IM, nb, cnt), MUL)
                    nc.vector.tensor_tensor(tv(t3, p0, L), fmb(RE, na, cnt),
                                            fmr(IM, nb, cnt), MUL)
                    nc.vector.tensor_tensor(tv(t4, p0, L), fmb(IM, na, cnt),
                                            fmr(RE, nb, cnt), MUL)
                nc.vector.tensor_tensor(full(t1), full(t1), full(t2), SUB)
                nc.vector.tensor_tensor(full(t3), full(t3), full(t4), ADD)
                # stage 2 (conj-side ACT pre-dup was tried: -6.6us DVE but
                # +35us Scalar, which delays psum-drain copies -> conv slips;
                # net loss, so keep the run-split muls on DVE)
                y = yp.tile([128, 16 * 256], F16, tag="y")
                yv_re = sview(y, 64, [[256, Rg], [32, 6], [1, 16]])
                yv_im = sview(y, 80, [[256, Rg], [32, 6], [1, 16]])
                for (p0, cnt, na, nb, ncn) in RUNS:
                    L = cnt * 16
                    nc.vector.tensor_tensor(tv(t2, p0, L), tv(t1, p0, L),
                                            fmr(RE, ncn, cnt), MUL)
                    nc.vector.tensor_tensor(tv(t4, p0, L), tv(t3, p0, L),
                                            fmr(IM, ncn, cnt), MUL)
                nc.vector.tensor_tensor(yv_re, full(t2), full(t4), ADD)
                for (p0, cnt, na, nb, ncn) in RUNS:
                    L = cnt * 16
                    nc.vector.tensor_tensor(tv(t2, p0, L), tv(t3, p0, L),
                                            fmr(RE, ncn, cnt), MUL)
                    nc.vector.tensor_tensor(tv(t4, p0, L), tv(t1, p0, L),
                                            fmr(IM, ncn, cnt), MUL)
                nc.vector.tensor_tensor(yv_im, full(t2), full(t4), SUB)
                # (0,n): y[16:64] = re0 * (re(n)^2 + im(n)^2), n=1..3
                # (GpSimd tensor_tensor measured ~5x slower than its cost
                # model here - keep squares on ACT, combines on DVE)
                s1 = tmpp.tile([128, 16 * 48], F16, tag="s1")
                s2 = tmpp.tile([128, 16 * 48], F16, tag="s2")
                nc.scalar.square(s1[0:WO, 0:Rg * 48], fmr(RE, 1, 3))
                nc.scalar.square(s2[0:WO, 0:Rg * 48], fmr(IM, 1, 3))
                nc.vector.tensor_tensor(s1[0:WO, 0:Rg * 48],
                                        s1[0:WO, 0:Rg * 48],
                                        s2[0:WO, 0:Rg * 48], ADD)
                nc.vector.tensor_tensor(
                    sview(y, 16, [[256, Rg], [1, 48]]),
                    s1[0:WO, 0:Rg * 48], fmb(RE, 0, 3), MUL)
                # (0,0): y[0:16] = re0^3
                s3 = tmpp.tile([128, 16 * 16], F16, tag="s3")
                nc.scalar.square(s3[0:WO, 0:Rg * 16], fmr(RE, 0, 1))
                nc.vector.tensor_tensor(
                    sview(y, 0, [[256, Rg], [1, 16]]),
                    s3[0:WO, 0:Rg * 16], fmr(RE, 0, 1), MUL)
                if late:
                    # leave the ACT table on Copy so teardown doesn't
                    # trigger a trailing 16KB table-load DMA
                    nc.scalar.copy(s3[0:1, 0:16], s2[0:1, 0:16])
                # bias (stride-0 bcast) + relu + output DMA
                bview = lambda n: sview(biasT, 0, [[0, n], [1, 256]])
                if not late:
                    yf = y[0:WO, 0:Rg * 256]
                    nc.vector.tensor_tensor(yf, yf, bview(Rg), ADD)
                    nc.vector.tensor_scalar_max(yf, yf, 0.0)
                    nc.gpsimd.dma_start(
                        _ap(out_d, h0 * 256,
                            [[HO * 256, WO], [1, Rg * 256]]),
                        yf)
                else:
                    # per-2-row chunks: pipeline relu with DMA drain.
                    # 2-row chunks spread across the DMA-engine fleet (big
                    # DMAs land on ~2 engines); rotate the 3 trigger queues.
                    # The very last chunk goes as two 1-row DMAs on two idle
                    # queues so the final drain halves. (Also tried: 4-row
                    # chunks, gpsimd-heavy and slow-queue-first patterns —
                    # all within noise or worse; drain is bound by per-engine
                    # descriptor rate after the last relu.)
                    engs = [nc.gpsimd, nc.scalar, nc.sync]
                    final = (h0 + Rg == HO)
                    for ci, j0 in enumerate(range(0, Rg, 2)):
                        yc = y[0:WO, j0 * 256:(j0 + 2) * 256]
                        nc.vector.tensor_tensor(yc, yc, bview(2), ADD)
                        nc.vector.tensor_scalar_max(yc, yc, 0.0)
                        if final and j0 == Rg - 2:
                            nc.scalar.dma_start(
                                _ap(out_d, (h0 + j0) * 256,
                                    [[HO * 256, WO], [1, 256]]),
                                y[0:WO, j0 * 256:(j0 + 1) * 256])
                            nc.sync.dma_start(
                                _ap(out_d, (h0 + j0 + 1) * 256,
                                    [[HO * 256, WO], [1, 256]]),
                                y[0:WO, (j0 + 1) * 256:(j0 + 2) * 256])
                        else:
                            engs[ci % 3].dma_start(
                                _ap(out_d, (h0 + j0) * 256,
                                    [[HO * 256, WO], [1, 2 * 256]]),
                                yc)

            # ---- stack DMA blocks (input rows -> (kw,c)-stacked SBUF) ----
            sA_t = {}
            sB_t = {}

            def emit_stack_block(r0, nr, tag, first=False):
                sAb = stkp.tile([128, nr * WO], F16, tag=f"sA{tag}",
                                name=f"sA{r0}")
                sBb = stkp.tile([96, nr * WO], F16, tag=f"sB{tag}",
                                name=f"sB{r0}")
                eng_b = nc.gpsimd if first else nc.sync
                for kw in range(4):
                    nc.sync.dma_start(
                        _ap(sAb[:], kw * 32 * (nr * WO),
                            [[nr * WO, 32], [WO, nr], [1, WO]]),
                        _ap(xt_d, r0 * C_IN * W + kw,
                            [[W, C_IN], [C_IN * W, nr], [1, WO]]))
                for kw in range(3):
                    eng_b.dma_start(
                        _ap(sBb[:], kw * 32 * (nr * WO),
                            [[nr * WO, 32], [WO, nr], [1, WO]]),
                        _ap(xt_d, r0 * C_IN * W + 4 + kw,
                            [[W, C_IN], [C_IN * W, nr], [1, WO]]))
                for j in range(nr):
                    sA_t[r0 + j] = (sAb, j)
                    sB_t[r0 + j] = (sBb, j)

            emit_stack_block(0, 4, "f", first=True)
            nc.scalar.dma_start(fA[:, HD:], fa_d[:, HD:])
            nc.scalar.dma_start(fB[:, HD:], fb_d[:, HD:])
            nc.scalar.dma_start(biasT[:], bias_d[:])
            for (r0, nr) in ((4, 4), (8, 8), (16, 8), (24, 8)):
                emit_stack_block(r0, nr, "f")
            for blk in range(1, 4):
                emit_stack_block(blk * 32, 32, "c")

            # ---- paired-psum conv ----
            psum_pair = {}

            def conv_row(r):
                sA = sA_t[r][0][:, sA_t[r][1] * WO:(sA_t[r][1] + 1) * WO]
                sB = sB_t[r][0][:, sB_t[r][1] * WO:(sB_t[r][1] + 1) * WO]
                if r % 2 == 1:
                    sAp = sA_t[r - 1][0][:, sA_t[r - 1][1] * WO:
                                         (sA_t[r - 1][1] + 1) * WO]
                    sBp = sB_t[r - 1][0][:, sB_t[r - 1][1] * WO:
                                         (sB_t[r - 1][1] + 1) * WO]
                # per pair g: psum cols [0:208]=row 2g+1, [208:416]=row 2g.
                # Pair's first touch is a full-width start=True matmul at
                # r=2g+1 (bank-wide psum reset); row 2g's kh=0 single is
                # deferred to just after it, using row r-1's stack.
                # entries: (g, col0, ncol, fcol0, start, use_prev_stack)
                mms = []
                stop_idx = -1
                if r % 2 == 0:
                    g = r // 2
                    if g < NPAIR:
                        psum_pair[g] = psp.tile([128, 416], F32, tag="ps",
                                                name=f"pp{g}")
                    for kp in (1, 3, 5):                         # fulls
                        g = (r - kp - 1) // 2
                        if 0 <= g < NPAIR:
                            mms.append((g, 0, 416, kp * NCONV, False, False))
                else:
                    # order singles between fulls: a single's 87ns stream
                    # can't hide the next LDWEIGHTS (135ns); a full's 173ns
                    # stream can. Batches end with a full for the same
                    # reason across the A->B and row transitions.
                    g0 = (r - 1) // 2
                    if g0 < NPAIR:                               # kh'=0,kh=1
                        mms.append((g0, 0, 416, 0, True, False))
                    if r >= 7:
                        g = (r - 7) // 2                         # kh'=6
                        mms.append((g, 0, 208, 6 * NCONV, False, False))
                        stop_idx = len(mms) - 1
                    g = (r - 3) // 2
                    if 0 <= g < NPAIR:
                        mms.append((g, 0, 416, 2 * NCONV, False, False))
                    if g0 < NPAIR:                               # kh=0 defer
                        mms.append((g0, 208, 208, 0, False, True))
                    g = (r - 5) // 2
                    if 0 <= g < NPAIR:
                        mms.append((g, 0, 416, 4 * NCONV, False, False))
                for (g, c0, ncol, f0, st, prev) in mms:
                    nc.tensor.matmul(psum_pair[g][0:WO, c0:c0 + ncol],
                                     sAp if prev else sA,
                                     fA[:, f0:f0 + ncol],
                                     start=st, stop=False)
                for i, (g, c0, ncol, f0, st, prev) in enumerate(mms):
                    nc.tensor.matmul(psum_pair[g][0:WO, c0:c0 + ncol],
                                     sBp if prev else sB,
                                     fB[:, f0:f0 + ncol],
                                     start=False, stop=(i == stop_idx))
                if r % 2 == 1 and r >= 7:
                    pair_complete((r - 7) // 2)

            def pair_complete(g):
                h = 2 * g            # rows h, h+1 done (stored swapped)
                gi, j = H2G[h]
                if j == 0:
                    group["fm"] = fmp.tile([128, 16 * NCONV], F16,
                                           tag="fm", name=f"fm{h}")
                    group["h0"] = h
                    group["Rg"] = GDEF[gi][1]
                    group["gi"] = gi
                ps = psum_pair.pop(g)
                with tc.high_priority():
                    nc.scalar.copy(
                        group["fm"][0:WO, j * NCONV:(j + 2) * NCONV],
                        ps[0:WO, 0:416])
                if j == group["Rg"] - 2:
                    bisp(group["fm"], group["h0"], group["Rg"],
                         late=(group["gi"] >= len(GDEF) - 2))

            for r in range(H):
                conv_row(r)
    nc.compile()
    return nc


def _get_program():
    global _PROGRAM
    if _PROGRAM is None:
        _PROGRAM = _build_program()
    return _PROGRAM


def _install_trace_shim():
    """antenv.axon_hooks is absent in this image; recreate via ctypes."""
    if "antenv.axon_hooks" in sys.modules:
        return
    try:
        from trn_agent_boot.trn_boot import _ntff_profile_via_ctypes
        hook = _ntff_profile_via_ctypes("/opt/axon/libaxon_pjrt.so")
    except Exception:
        hook = None
    m = types.ModuleType("antenv.axon_hooks")
    m.get_axon_ntff_profile_hook = lambda: hook
    m.set_axon_ntff_profile_hook = lambda h: None
    sys.modules["antenv.axon_hooks"] = m
    bass_utils.upload_artifacts = lambda tmpdir: tmpdir


_PERM = np.arange(HO).reshape(-1, 2)[:, ::-1].reshape(-1)  # pair swap


def kernel(x, w, bias, _trace=False, _tmpdir=None):
    """Full inputs -> full output (8,122,122,256) float32."""
    x = np.asarray(x, dtype=np.float32)
    w = np.asarray(w, dtype=np.float32)
    bias = np.asarray(bias, dtype=np.float32)

    fa, fb = _host_filters(w)
    biasrep = np.broadcast_to(bias.astype(np.float16)[None, :],
                              (128, 256)).copy()
    in_maps = []
    for b in range(NB):
        xt = np.ascontiguousarray(x[b].transpose(0, 2, 1)).astype(np.float16)
        in_maps.append({"xt": xt, "fa": fa, "fb": fb, "biasrep": biasrep})

    nc = _get_program()
    kwargs = {}
    if _trace:
        _install_trace_shim()
        kwargs = dict(trace=True, tmpdir=_tmpdir)
    res = bass_utils.run_bass_kernel_spmd(nc, in_maps,
                                          core_ids=list(range(NB)), **kwargs)
    out = np.stack([res.results[b]["out"].transpose(1, 0, 2)[_PERM]
                    for b in range(NB)], axis=0).astype(np.float32)
    if _trace:
        return out, res
    return out


if __name__ == "__main__":
    d = np.load("/tmp/ref_io.npz")
    out = kernel(d["x"], d["w"], d["bias"])
    exp = d["expected"]
    rel = np.linalg.norm(out - exp) / np.linalg.norm(exp)
    print("rel_l2 =", rel)


# revision 43
# speedup vs baseline: 1.0280x; 1.0280x over previous
"""Trainium2 Bass kernel for nn_BCHConv2D (complex harmonic conv + bispectrum).

Strategy (8 NeuronCores, data-parallel over batch B=8):
  host: build complex-harmonic filters from w+atoms -> fdA [128, 7*208],
        fdB [96, 7*208] (kh-major along free dim); transpose each batch
        image to (H, C, W); un-swap output row pairs at the end.
  core, conv: 7x7x32 -> 208ch conv with ROW-PAIRED psum banks: one bank
        holds two adjacent output rows [122, 2*208]; a single matmul
        streams a 416-col moving slice fd[kh-1 | kh] accumulating both
        rows at once (8 matmuls + 8 LDWEIGHTS per pair-side instead of
        14 per-row), with a full-width start=True matmul first (psum
        start resets the whole bank) and the even row's kh=0 single
        deferred after it. PE warms up ~28 matmuls on random SBUF data
        immediately (no DMA dependency; HAM pstate ramps while the
        filter/stack DMAs land). Conv runs at ~97% of its structural
        floor (14x208 stream cols/row, fp16 1 cyc/row).
  core, bisp: PSUM pair -> fp16 SBUF copy (one ACT copy per pair) ->
        bispectrum as blocked DVE ops (+ ACT squares) over ~10-row
        groups (schedule-DP optimum: small groups keep the DVE fed as
        conv rows arrive, shrinking the after-conv Vector backlog) ->
        stride-0-broadcast bias add + relu -> output DMA.
  DMA:  input stacks on sync; outputs on gpsimd (its software DGE
        spreads DRAM-write descriptors across the engine fleet; HW
        queues pin writes to ~2 engines at ~11GB/s). Last two groups
        stream out in 2-row chunks, the final chunk as two 1-row DMAs
        on idle queues so the end-of-kernel drain is short.
"""
import os
import sys
import types
from itertools import product

import numpy as np

sys.path.insert(0, "/opt/trn_rl_repo")
sys.path.insert(0, "/root/.axon_site")

import concourse.bass as bass
import concourse.bacc as bacc
import concourse.tile as tile
from concourse import mybir
from concourse import bass_utils

# ---------------- problem constants ----------------
KS, MD, STREAMS, C_IN = 7, 6, 16, 32
H = W = 128
HO = WO = 122
NC_RE = (MD + 1) * STREAMS       # 112
NCONV = 208                      # re[0..6] (112) + im[1..6] (96); im0 == 0
IM_BASE = NC_RE - 16             # im[n] at IM_BASE + n*16 for n >= 1
NB = 8                           # batch == cores
NPAIR = HO // 2                  # 61 row pairs

F16 = mybir.dt.float16
F32 = mybir.dt.float32

NWARM = int(os.environ.get("NWARM", "28"))


# ---------------- host-side filter construction ----------------
def _tri(v):
    return np.where(np.abs(v) <= 1, np.where(v < 0, v + 1, 1 - v), 0)


def _make_atoms(kernel_size, max_degree):
    radius = (kernel_size - 1) // 2
    g = np.arange(-radius, radius + 1)
    xg, yg = np.meshgrid(g, g)
    r = np.sqrt(xg ** 2 + yg ** 2)
    theta = np.arctan2(yg, xg)
    n_rp = kernel_size // 2 + 1
    atoms = np.zeros((kernel_size, kernel_size, max_degree + 1, n_rp),
                     dtype=np.complex64)
    for i, n in product(range(n_rp), range(max_degree + 1)):
        atoms[:, :, n, i] = _tri(r - i) * np.exp(theta * n * -1j)
    atoms[kernel_size // 2, kernel_size // 2, 1:, :] = 0
    norm = np.sqrt(np.sum(np.conj(atoms) * atoms, axis=(0, 1)))
    norm[norm == 0] = 1
    return (atoms / norm).astype(np.complex64)


_ATOMS = _make_atoms(KS, MD)


def _host_filters(w):
    """w (1,1,32,16,7,4) -> (fdA [128, 7*208], fdB [96, 7*208]) float16.
    Conv channel order: col n*16+s = re(n,s); 112+(n-1)*16+s = im(n,s)."""
    wc = w[0, 0]
    f_re = np.einsum("hwnr,csnr->hwcsn", _ATOMS.real, wc)
    f_im = np.einsum("hwnr,csnr->hwcsn", _ATOMS.imag, wc)
    filt = np.zeros((KS, KS, C_IN, NCONV), np.float32)
    filt[:, :, :, 0:NC_RE] = np.transpose(f_re, (0, 1, 2, 4, 3)).reshape(
        KS, KS, C_IN, NC_RE)
    filt[:, :, :, NC_RE:NCONV] = np.transpose(
        f_im[:, :, :, :, 1:], (0, 1, 2, 4, 3)).reshape(KS, KS, C_IN, 96)
    # [kh, (kw,c), o] -> partition-major stacks, kh-major along free dim
    fa = filt[:, 0:4].reshape(KS, 128, NCONV).transpose(1, 0, 2).reshape(
        128, KS * NCONV)
    fb = filt[:, 4:7].reshape(KS, 96, NCONV).transpose(1, 0, 2).reshape(
        96, KS * NCONV)
    return fa.astype(np.float16), fb.astype(np.float16)


# ---------------- bass program ----------------
def _ap(src_ap, off, dims):
    """New AP into the same tensor: explicit [step, count] dims (elements)."""
    return bass.AP(tensor=src_ap.tensor, offset=src_ap.offset + off, ap=dims)


_PROGRAM = None

# bisp groups in row units: (h0, Rg); all starts/sizes even.
# ~10-row groups keep the DVE fed as conv rows arrive (schedule-DP
# optimum): the final group starts at conv-end either way, so smaller
# mid-kernel groups shrink the end backlog.
GDEF = [(0, 12)] + [(12 + 10 * k, 10) for k in range(11)]


def _build_program():
    nc = bacc.Bacc("TRN2", target_bir_lowering=False, debug=False,
                   num_devices=NB)
    xt_d = nc.dram_tensor("xt", [H, C_IN, W], F16, kind="ExternalInput").ap()
    fa_d = nc.dram_tensor("fa", [128, KS * NCONV], F16,
                          kind="ExternalInput").ap()
    fb_d = nc.dram_tensor("fb", [96, KS * NCONV], F16,
                          kind="ExternalInput").ap()
    bias_d = nc.dram_tensor("biasrep", [128, 256], F16,
                            kind="ExternalInput").ap()
    # w-major output layout: row pairs stored swapped (host un-swaps)
    out_d = nc.dram_tensor("out", [WO, HO, 256], F16,
                           kind="ExternalOutput").ap()
    MUL = mybir.AluOpType.mult
    ADD = mybir.AluOpType.add
    SUB = mybir.AluOpType.subtract

    with tile.TileContext(nc) as tc:
        with tc.tile_pool(name="const", bufs=1) as constp, \
             tc.tile_pool(name="stk", bufs=5) as stkp, \
             tc.tile_pool(name="fm", bufs=4) as fmp, \
             tc.tile_pool(name="tmp", bufs=2) as tmpp, \
             tc.tile_pool(name="yp", bufs=4) as yp, \
             tc.tile_pool(name="ps", bufs=8, space="PSUM") as psp:

            # ---- PE warmup on random SBUF data: no DMA dependency ----
            wtile = constp.tile([128, 256], F16, name="warm")
            nc.vector.random(wtile[:])
            wps = psp.tile([128, 416], F32, tag="ps", name="warmps")
            for _ in range(NWARM):
                nc.tensor.matmul(wps[0:122, 0:208], wtile[:, 0:122],
                                 wtile[:, 0:208], start=True, stop=True)

            # ---- constants (DMAs spread across queues) ----
            fA = constp.tile([128, KS * NCONV], F16, name="fA")
            fB = constp.tile([96, KS * NCONV], F16, name="fB")
            # (filter-head split + shorter warmup was tried: conv rows >=2
            # stall on the late fd tails, ~+5us. Keep full halves up front.)
            HK = KS * NCONV // 2  # 728
            nc.scalar.dma_start(fA[:, 0:HK], fa_d[:, 0:HK])
            nc.gpsimd.dma_start(fA[:, HK:], fa_d[:, HK:])
            nc.scalar.dma_start(fB[:, 0:HK], fb_d[:, 0:HK])
            nc.gpsimd.dma_start(fB[:, HK:], fb_d[:, HK:])
            biasT = constp.tile([128, 256], F16)
            nc.scalar.dma_start(biasT[:], bias_d[:])

            group = {}   # current bispectrum group state
            H2G = {}
            for gi, (s, n) in enumerate(GDEF):
                for j in range(n):
                    H2G[s + j] = (gi, j)

            def sview(t, off, dims, nparts=WO):
                a = t[:]
                return bass.AP(tensor=a.tensor, offset=a.offset + off,
                               ap=[[a.ap[0][0], nparts]] + dims)

            def bisp(fmT, h0, Rg, late=False):
                """Bispectrum for Rg rows; fm blocks hold rows pair-swapped
                (host un-swaps). Reads via run-decomposed broadcast APs."""
                def fmr(comp, n0, cnt):     # contiguous n-run view
                    return sview(fmT, comp + n0 * 16,
                                 [[NCONV, Rg], [1, cnt * 16]])

                def fmb(comp, n, reps):     # broadcast single-n view
                    return sview(fmT, comp + n * 16,
                                 [[NCONV, Rg], [0, reps], [1, 16]])

                t1 = tmpp.tile([128, 16 * 96], F16, tag="t1")
                t2 = tmpp.tile([128, 16 * 96], F16, tag="t2")
                t3 = tmpp.tile([128, 16 * 96], F16, tag="t3")
                t4 = tmpp.tile([128, 16 * 96], F16, tag="t4")

                def tv(t, p0, L):
                    return sview(t, p0 * 16, [[96, Rg], [1, L]])
                full = lambda t: t[0:WO, 0:Rg * 96]
                RE, IM = 0, IM_BASE
                # runs: (pair0, cnt, nA, nB0, nC0)
                RUNS = [(0, 3, 1, 1, 2), (3, 2, 2, 2, 4), (5, 1, 3, 3, 6)]
                # stage 1: re1 = ArBr - AiBi -> t1 ; im1 = ArBi + AiBr -> t3
                for (p0, cnt, na, nb, ncn) in RUNS:
                    L = cnt * 16
                    nc.vector.tensor_tensor(tv(t1, p0, L), fmb(RE, na, cnt),
                                            fmr(RE, nb, cnt), MUL)
                    nc.vector.tensor_tensor(tv(t2, p0, L), fmb(IM, na, cnt),
                                            fmr(IM, nb, cnt), MUL)
                    nc.vector.tensor_tensor(tv(t3, p0, L), fmb(RE, na, cnt),
                                            fmr(IM, nb, cnt), MUL)
                    nc.vector.tensor_tensor(tv(t4, p0, L), fmb(IM, na, cnt),
                                            fmr(RE, nb, cnt), MUL)
                nc.vector.tensor_tensor(full(t1), full(t1), full(t2), SUB)
                nc.vector.tensor_tensor(full(t3), full(t3), full(t4), ADD)
                # stage 2 (conj-side ACT pre-dup was tried: -6.6us DVE but
                # +35us Scalar, which delays psum-drain copies -> conv slips;
                # net loss, so keep the run-split muls on DVE)
                y = yp.tile([128, 16 * 256], F16, tag="y")
                yv_re = sview(y, 64, [[256, Rg], [32, 6], [1, 16]])
                yv_im = sview(y, 80, [[256, Rg], [32, 6], [1, 16]])
                for (p0, cnt, na, nb, ncn) in RUNS:
                    L = cnt * 16
                    nc.vector.tensor_tensor(tv(t2, p0, L), tv(t1, p0, L),
                                            fmr(RE, ncn, cnt), MUL)
                    nc.vector.tensor_tensor(tv(t4, p0, L), tv(t3, p0, L),
                                            fmr(IM, ncn, cnt), MUL)
                nc.vector.tensor_tensor(yv_re, full(t2), full(t4), ADD)
                for (p0, cnt, na, nb, ncn) in RUNS:
                    L = cnt * 16
                    nc.vector.tensor_tensor(tv(t2, p0, L), tv(t3, p0, L),
                                            fmr(RE, ncn, cnt), MUL)
                    nc.vector.tensor_tensor(tv(t4, p0, L), tv(t1, p0, L),
                                            fmr(IM, ncn, cnt), MUL)
                nc.vector.tensor_tensor(yv_im, full(t2), full(t4), SUB)
                # (0,n): y[16:64] = re0 * (re(n)^2 + im(n)^2), n=1..3
                # (GpSimd tensor_tensor measured ~5x slower than its cost
                # model here - keep squares on ACT, combines on DVE)
                s1 = tmpp.tile([128, 16 * 48], F16, tag="s1")
                s2 = tmpp.tile([128, 16 * 48], F16, tag="s2")
                nc.scalar.square(s1[0:WO, 0:Rg * 48], fmr(RE, 1, 3))
                nc.scalar.square(s2[0:WO, 0:Rg * 48], fmr(IM, 1, 3))
                nc.vector.tensor_tensor(s1[0:WO, 0:Rg * 48],
                                        s1[0:WO, 0:Rg * 48],
                                        s2[0:WO, 0:Rg * 48], ADD)
                nc.vector.tensor_tensor(
                    sview(y, 16, [[256, Rg], [1, 48]]),
                    s1[0:WO, 0:Rg * 48], fmb(RE, 0, 3), MUL)
                # (0,0): y[0:16] = re0^3
                s3 = tmpp.tile([128, 16 * 16], F16, tag="s3")
                nc.scalar.square(s3[0:WO, 0:Rg * 16], fmr(RE, 0, 1))
                nc.vector.tensor_tensor(
                    sview(y, 0, [[256, Rg], [1, 16]]),
                    s3[0:WO, 0:Rg * 16], fmr(RE, 0, 1), MUL)
                if late:
                    # leave the ACT table on Copy so teardown doesn't
                    # trigger a trailing 16KB table-load DMA
                    nc.scalar.copy(s3[0:1, 0:16], s2[0:1, 0:16])
                # bias (stride-0 bcast) + relu + output DMA
                bview = lambda n: sview(biasT, 0, [[0, n], [1, 256]])
                if not late:
                    yf = y[0:WO, 0:Rg * 256]
                    nc.vector.tensor_tensor(yf, yf, bview(Rg), ADD)
                    nc.vector.tensor_scalar_max(yf, yf, 0.0)
                    nc.gpsimd.dma_start(
                        _ap(out_d, h0 * 256,
                            [[HO * 256, WO], [1, Rg * 256]]),
                        yf)
                else:
                    # per-2-row chunks: pipeline relu with DMA drain.
                    # 2-row chunks spread across the DMA-engine fleet (big
                    # DMAs land on ~2 engines); rotate the 3 trigger queues.
                    # The very last chunk goes as two 1-row DMAs on two idle
                    # queues so the final drain halves. (Also tried: 4-row
                    # chunks, gpsimd-heavy and slow-queue-first patterns —
                    # all within noise or worse; drain is bound by per-engine
                    # descriptor rate after the last relu.)
                    engs = [nc.gpsimd, nc.scalar, nc.sync]
                    final = (h0 + Rg == HO)
                    for ci, j0 in enumerate(range(0, Rg, 2)):
                        yc = y[0:WO, j0 * 256:(j0 + 2) * 256]
                        nc.vector.tensor_tensor(yc, yc, bview(2), ADD)
                        nc.vector.tensor_scalar_max(yc, yc, 0.0)
                        if final and j0 == Rg - 2:
                            nc.scalar.dma_start(
                                _ap(out_d, (h0 + j0) * 256,
                                    [[HO * 256, WO], [1, 256]]),
                                y[0:WO, j0 * 256:(j0 + 1) * 256])
                            nc.sync.dma_start(
                                _ap(out_d, (h0 + j0 + 1) * 256,
                                    [[HO * 256, WO], [1, 256]]),
                                y[0:WO, (j0 + 1) * 256:(j0 + 2) * 256])
                        else:
                            engs[ci % 3].dma_start(
                                _ap(out_d, (h0 + j0) * 256,
                                    [[HO * 256, WO], [1, 2 * 256]]),
                                yc)

            # ---- stack DMA blocks (input rows -> (kw,c)-stacked SBUF) ----
            sA_t = {}
            sB_t = {}

            def emit_stack_block(r0, nr, tag, first=False):
                sAb = stkp.tile([128, nr * WO], F16, tag=f"sA{tag}",
                                name=f"sA{r0}")
                sBb = stkp.tile([96, nr * WO], F16, tag=f"sB{tag}",
                                name=f"sB{r0}")
                eng_b = nc.gpsimd if first else nc.sync
                for kw in range(4):
                    nc.sync.dma_start(
                        _ap(sAb[:], kw * 32 * (nr * WO),
                            [[nr * WO, 32], [WO, nr], [1, WO]]),
                        _ap(xt_d, r0 * C_IN * W + kw,
                            [[W, C_IN], [C_IN * W, nr], [1, WO]]))
                for kw in range(3):
                    eng_b.dma_start(
                        _ap(sBb[:], kw * 32 * (nr * WO),
                            [[nr * WO, 32], [WO, nr], [1, WO]]),
                        _ap(xt_d, r0 * C_IN * W + 4 + kw,
                            [[W, C_IN], [C_IN * W, nr], [1, WO]]))
                for j in range(nr):
                    sA_t[r0 + j] = (sAb, j)
                    sB_t[r0 + j] = (sBb, j)

            emit_stack_block(0, 4, "f", first=True)
            for (r0, nr) in ((4, 4), (8, 8), (16, 8), (24, 8)):
                emit_stack_block(r0, nr, "f")
            for blk in range(1, 4):
                emit_stack_block(blk * 32, 32, "c")

            # ---- paired-psum conv ----
            psum_pair = {}

            def conv_row(r):
                sA = sA_t[r][0][:, sA_t[r][1] * WO:(sA_t[r][1] + 1) * WO]
                sB = sB_t[r][0][:, sB_t[r][1] * WO:(sB_t[r][1] + 1) * WO]
                if r % 2 == 1:
                    sAp = sA_t[r - 1][0][:, sA_t[r - 1][1] * WO:
                                         (sA_t[r - 1][1] + 1) * WO]
                    sBp = sB_t[r - 1][0][:, sB_t[r - 1][1] * WO:
                                         (sB_t[r - 1][1] + 1) * WO]
                # per pair g: psum cols [0:208]=row 2g+1, [208:416]=row 2g.
                # Pair's first touch is a full-width start=True matmul at
                # r=2g+1 (bank-wide psum reset); row 2g's kh=0 single is
                # deferred to just after it, using row r-1's stack.
                # entries: (g, col0, ncol, fcol0, start, use_prev_stack)
                mms = []
                stop_idx = -1
                if r % 2 == 0:
                    g = r // 2
                    if g < NPAIR:
                        psum_pair[g] = psp.tile([128, 416], F32, tag="ps",
                                                name=f"pp{g}")
                    for kp in (1, 3, 5):                         # fulls
                        g = (r - kp - 1) // 2
                        if 0 <= g < NPAIR:
                            mms.append((g, 0, 416, kp * NCONV, False, False))
                else:
                    # order singles between fulls: a single's 87ns stream
                    # can't hide the next LDWEIGHTS (135ns); a full's 173ns
                    # stream can. Batches end with a full for the same
                    # reason across the A->B and row transitions.
                    g0 = (r - 1) // 2
                    if g0 < NPAIR:                               # kh'=0,kh=1
                        mms.append((g0, 0, 416, 0, True, False))
                    if r >= 7:
                        g = (r - 7) // 2                         # kh'=6
                        mms.append((g, 0, 208, 6 * NCONV, False, False))
                        stop_idx = len(mms) - 1
                    g = (r - 3) // 2
                    if 0 <= g < NPAIR:
                        mms.append((g, 0, 416, 2 * NCONV, False, False))
                    if g0 < NPAIR:                               # kh=0 defer
                        mms.append((g0, 208, 208, 0, False, True))
                    g = (r - 5) // 2
                    if 0 <= g < NPAIR:
                        mms.append((g, 0, 416, 4 * NCONV, False, False))
                for (g, c0, ncol, f0, st, prev) in mms:
                    nc.tensor.matmul(psum_pair[g][0:WO, c0:c0 + ncol],
                                     sAp if prev else sA,
                                     fA[:, f0:f0 + ncol],
                                     start=st, stop=False)
                for i, (g, c0, ncol, f0, st, prev) in enumerate(mms):
                    nc.tensor.matmul(psum_pair[g][0:WO, c0:c0 + ncol],
                                     sBp if prev else sB,
                                     fB[:, f0:f0 + ncol],
                                     start=False, stop=(i == stop_idx))
                if r % 2 == 1 and r >= 7:
                    pair_complete((r - 7) // 2)

            def pair_complete(g):
                h = 2 * g            # rows h, h+1 done (stored swapped)
                gi, j = H2G[h]
                if j == 0:
                    group["fm"] = fmp.tile([128, 16 * NCONV], F16,
                                           tag="fm", name=f"fm{h}")
                    group["h0"] = h
                    group["Rg"] = GDEF[gi][1]
                    group["gi"] = gi
                ps = psum_pair.pop(g)
                with tc.high_priority():
                    nc.scalar.copy(
                        group["fm"][0:WO, j * NCONV:(j + 2) * NCONV],
                        ps[0:WO, 0:416])
                if j == group["Rg"] - 2:
                    bisp(group["fm"], group["h0"], group["Rg"],
                         late=(group["gi"] >= len(GDEF) - 2))

            for r in range(H):
                conv_row(r)
    nc.compile()
    return nc


def _get_program():
    global _PROGRAM
    if _PROGRAM is None:
        _PROGRAM = _build_program()
    return _PROGRAM


def _install_trace_shim():
    """antenv.axon_hooks is absent in this image; recreate via ctypes."""
    if "antenv.axon_hooks" in sys.modules:
        return
    try:
        from trn_agent_boot.trn_boot import _ntff_profile_via_ctypes
        hook = _ntff_profile_via_ctypes("/opt/axon/libaxon_pjrt.so")
    except Exception:
        hook = None
    m = types.ModuleType("antenv.axon_hooks")
    m.get_axon_ntff_profile_hook = lambda: hook
    m.set_axon_ntff_profile_hook = lambda h: None
    sys.modules["antenv.axon_hooks"] = m
    bass_utils.upload_artifacts = lambda tmpdir: tmpdir


_PERM = np.arange(HO).reshape(-1, 2)[:, ::-1].reshape(-1)  # pair swap


def kernel(x, w, bias, _trace=False, _tmpdir=None):
    """Full inputs -> full output (8,122,122,256) float32."""
    x = np.asarray(x, dtype=np.float32)
    w = np.asarray(w, dtype=np.float32)
    bias = np.asarray(bias, dtype=np.float32)

    fa, fb = _host_filters(w)
    biasrep = np.broadcast_to(bias.astype(np.float16)[None, :],
                              (128, 256)).copy()
    in_maps = []
    for b in range(NB):
        xt = np.ascontiguousarray(x[b].transpose(0, 2, 1)).astype(np.float16)
        in_maps.append({"xt": xt, "fa": fa, "fb": fb, "biasrep": biasrep})

    nc = _get_program()
    kwargs = {}
    if _trace:
        _install_trace_shim()
        kwargs = dict(trace=True, tmpdir=_tmpdir)
    res = bass_utils.run_bass_kernel_spmd(nc, in_maps,
                                          core_ids=list(range(NB)), **kwargs)
    out = np.stack([res.results[b]["out"].transpose(1, 0, 2)[_PERM]
                    for b in range(NB)], axis=0).astype(np.float32)
    if _trace:
        return out, res
    return out


if __name__ == "__main__":
    d = np.load("/tmp/ref_io.npz")
    out = kernel(d["x"], d["w"], d["bias"])
    exp = d["expected"]
    rel = np.linalg.norm(out - exp) / np.linalg.norm(exp)
    print("rel_l2 =", rel)
